# revision 39
# baseline (speedup 1.0000x reference)
"""Trainium2 Bass kernel for single-head attention (nn_MultiHeadAttention).

Reference computation (B=4, S=2048, D=1024, fp32):
    K = _K @ Wk.T + bk ; V = _V @ Wv.T + bv ; Q = _Q @ Wq.T + bq
    scores[b,k,q] = (K[b,k,:] . Q[b,q,:]) / sqrt(D)
    alpha = softmax(scores, axis=keys)
    V_[b,q,:] = sum_k V[b,k,:] * alpha[b,k,q]
    O = V_ @ Wo.T + bo

Sharding: core c = (b, h) with b = c//2 (batch), h = c%2 (query half of
1024). Each core handles the full key/value sequence of its batch and a
1024-query slice. K/V projections are split across the pair (each core
projects its half of the keys) and exchanged with a pair-wise AllGather.

Device-side layout strategy (per core):
  - Host pre-transposes activations/weights so every matmul contraction
    dim lands on SBUF partitions: _K.T/_V.T/_Q.T as [d, s], W.T as [d, e].
  - Projections produce K.T and Q.T as [e, s] (feature on partitions) and
    V naturally as [k, e]; scores = K.T' @ Q.T gives [k, q] tiles.
  - Softmax over keys (the partition dim) avoids a partition reduction:
    exp(scores/32) is taken unstabilized (scores ~ N(0,1), max << 88) and
    the key-sums are computed with an all-ones stationary matmul, which
    broadcasts sum_k es[k,q] across all 128 partitions.
  - Normalization is deferred: unnormalized V.T@es = [e, q] tiles are
    scaled by 1/sum (free-dim aligned thanks to the broadcast trick), then
    the output projection consumes them as stationary operands.
All matmuls are bf16 (M=128, N=512) accumulating in fp32 PSUM.

Schedule: a DMA trigger's semaphore wait blocks the whole issuing
engine's instruction queue, so DMAs are routed by wait profile:
  - Sync HWDGE: half the input streaming (parity-split with scalar), V
    staging, K + V copy-backs (their collective waits park only in front
    of the phase-D output stores), output stores.
  - Scalar HWDGE: the other half of the input streaming and K staging
    (waits per-block projection completion) — nothing that waits on a
    collective, so ScalarE is free for phase B's exp.
  - GpSimd: collective doorbells and the pair-sum subtracts only (the
    tile scheduler interleaves queue entries by its own priority, so an
    op waiting on a collective would poison the DVE queue).
The warmup collective carries no input dependency so the one-time comm
init barrier runs concurrently with input loads + K projection. Phases
run K -> V -> Q so both collectives overlap projection compute. A short
burst of dummy matmuls during the initial DMA fill keeps the PE HAM
clock-gate warm before the first real matmul.

Pair exchange is an AllReduce(add) of the projected halves, not an
AllGather: each core keeps its own half in SBUF (key order per core is
[own half, partner half] — attention is permutation-invariant over keys
as long as K and V agree), copies the 2MB sum back, and recovers the
partner half as (sum - own) on the Vector engine. This keeps every
address rank-independent (the program is SPMD), halves copy-back
traffic, and detaches phase B's first 8 key blocks from the collective
entirely, so comm latency variance is absorbed. Cost: one extra bf16
rounding on the partner half (~0.4% elementwise), well inside the
tolerance.
"""

import sys

if "/opt/trn_rl_repo" not in sys.path:
    sys.path.insert(0, "/opt/trn_rl_repo")

import ml_dtypes
import numpy as np

import concourse.bass as bass
import concourse.tile as tile
from concourse import bacc, mybir
from concourse.bass_utils import run_bass_kernel_spmd

B, S, D = 4, 2048, 1024
SQ = 1024  # queries per core
SH = 1024  # keys projected per core (half of S; pair AllGather fills the rest)
P = 128  # partitions
CH = 512  # matmul moving free dim (one fp32 PSUM bank)
EB = D // P  # 8 feature blocks
DB = D // P  # 8 contraction blocks
KB = S // P  # 16 key blocks
QB = SQ // P  # 8 query blocks
KC = S // CH  # 4 key chunks
QC = SQ // CH  # 2 query chunks
FC = D // CH  # 2 output-feature chunks
SCALE = 1.0 / np.sqrt(np.float32(D))  # folded into exp()

F32 = mybir.dt.float32
BF16 = mybir.dt.bfloat16
AF = mybir.ActivationFunctionType
NPBF16 = ml_dtypes.bfloat16

# test.py can flip this to get a profiled run; the measured NEFF time (max
# over traced cores) lands in LAST_EXEC_NS.
TRACE = False
TRACE_ALL_CORES = False
TRACE_TMPDIR = None  # test-only: where NTFF/perfetto artifacts land
LAST_EXEC_NS = None

_NC_CACHE = None


def _build_nc() -> bass.Bass:
    # Bacc (not plain Bass): its finalize() pipeline splits multi-sem waits
    # into event-semaphore chains — TRN2 instructions take at most 1 wait.
    nc = bacc.Bacc(num_devices=8)

    kt_d = nc.dram_tensor("kt", [D, SH], BF16, kind="ExternalInput")
    vt_d = nc.dram_tensor("vt", [D, SH], BF16, kind="ExternalInput")
    qt_d = nc.dram_tensor("qt", [D, SQ], BF16, kind="ExternalInput")
    wkt_d = nc.dram_tensor("wkt", [D, D], BF16, kind="ExternalInput")
    wqt_d = nc.dram_tensor("wqt", [D, D], BF16, kind="ExternalInput")
    wvt_d = nc.dram_tensor("wvt", [D, D], BF16, kind="ExternalInput")
    wot_d = nc.dram_tensor("wot", [D, D], BF16, kind="ExternalInput")
    bk_d = nc.dram_tensor("bk", [P, EB], F32, kind="ExternalInput")
    bq_d = nc.dram_tensor("bq", [P, EB], F32, kind="ExternalInput")
    bvb_d = nc.dram_tensor("bvb", [P, D], F32, kind="ExternalInput")
    bob_d = nc.dram_tensor("bob", [P, D], F32, kind="ExternalInput")
    o_d = nc.dram_tensor("o", [SQ, D], F32, kind="ExternalOutput")

    with tile.TileContext(nc) as tc:
        # Layout rule: regions that DMA ever lands in (weights, input
        # streams) are never reused by a later pool — a fresh tile in a
        # DMA-recycled region would carry a WAR wait on every HW DMA queue
        # and blow the per-instruction sync wait-table limit (8) in walrus.
        # Only wa (released, region then left dead) and kqt (ACT-written
        # es / DVE-written vtu recycle it) are ever released mid-kernel.
        p_misc = tc.alloc_tile_pool(name="misc", bufs=1, side="left")
        p_wo = tc.alloc_tile_pool(name="wo", bufs=1, side="left")
        p_ps = tc.alloc_tile_pool(name="ps", bufs=6, space="PSUM")
        p_pss = tc.alloc_tile_pool(name="pss", bufs=2, space="PSUM")
        p_v = tc.alloc_tile_pool(name="v", bufs=1, side="right")
        p_xk = tc.alloc_tile_pool(name="xk", bufs=1, side="right")
        p_xq = tc.alloc_tile_pool(name="xq", bufs=1, side="right")
        p_xv = tc.alloc_tile_pool(name="xv", bufs=1, side="right")
        p_kqt = tc.alloc_tile_pool(name="kqt", bufs=1, side="left")
        p_wa = tc.alloc_tile_pool(name="wa", bufs=1, side="left")

        p_dram = tc.alloc_tile_pool(name="dram", bufs=1, space="DRAM")

        dma = nc.sync.dma_start  # input/output streaming ring
        sdma = nc.scalar.dma_start  # collective staging + K copy-back ring

        recip_sb = p_misc.tile([P, SQ], F32)

        # Each core projects only its half of the keys; pair-wise AllGather
        # ({2b, 2b+1} share batch b; rank order = k order) fills the rest.
        # The first collective pays a large one-time comm-init cost, so a
        # dependency-free warmup gather (garbage data) is issued before
        # anything else and initializes the channels while inputs stream.
        CC_GROUPS = [[0, 1], [2, 3], [4, 5], [6, 7]]
        warm_in = p_dram.tile([1, 64], BF16)
        warm_out = p_dram.tile([2, 64], BF16)
        nc.gpsimd.collective_compute(
            "AllGather",
            mybir.AluOpType.bypass,
            replica_groups=CC_GROUPS,
            ins=[warm_in.opt()],
            outs=[warm_out.opt()],
        )
        cc_kin = p_dram.tile([D, SH], BF16)
        cc_ksum = p_dram.tile([D, SH], BF16)
        cc_vin = p_dram.tile([SH, D], BF16)
        cc_vsum = p_dram.tile([SH, D], BF16)

        # HAM pre-warm: the PE clock-gate needs ~3.4us of sustained matmul
        # activity to reach full rate. The initial input-DMA fill is ~10us
        # of forced PE idle; burn it on dummy matmuls so the projections
        # start warm.
        # gpsimd memset: its preamble retires ~2us before DVE's, so the
        # dummy burst starts that much earlier
        dum_sb = p_misc.tile([P, CH], BF16)
        nc.gpsimd.memset(dum_sb[:], 0.125)
        dum_ps = p_ps.tile([P, CH], F32, tag="ps", name="ps")
        for _ in range(16):
            nc.tensor.matmul(
                dum_ps[:], dum_sb[:, 0:P], dum_sb[:], start=True, stop=True
            )

        ones_sb = p_misc.tile([P, P], BF16)
        nc.vector.memset(ones_sb[:], 1.0)

        # ---- input streaming (both rings, in consumption order) ----
        bk_sb = p_misc.tile([P, EB], F32)
        dma(out=bk_sb[:], in_=bk_d[:])
        bq_sb = p_misc.tile([P, EB], F32)
        dma(out=bq_sb[:], in_=bq_d[:])

        wkt_sb = p_wa.tile([P, DB, D], BF16, name="wkt_sb")
        wqt_sb = p_wa.tile([P, DB, D], BF16, name="wqt_sb")
        wvt_sb = p_wa.tile([P, DB, D], BF16, name="wvt_sb")
        wot_sb = p_wo.tile([P, DB, D], BF16, name="wot_sb")
        xt_k = p_xk.tile([P, DB, SH], BF16)  # _K.T input: [d_p, d_blk, k]
        xt_q = p_xq.tile([P, DB, SQ], BF16)  # _Q.T input: [d_p, d_blk, q]
        xt_v = p_xv.tile([P, DB, SH], BF16)  # _V.T input: [d_p, d_blk, k]

        wkt_src = wkt_d.rearrange("(a p) e -> p a e", p=P)
        wqt_src = wqt_d.rearrange("(a p) e -> p a e", p=P)
        wvt_src = wvt_d.rearrange("(a p) e -> p a e", p=P)
        wot_src = wot_d.rearrange("(a p) e -> p a e", p=P)

        # Initial fill split across both HWDGE rings, parity-alternated so
        # each (weight[d], activation[d]) pair lands at the aggregate HBM
        # rate — neither ring becomes the single-file bottleneck for the
        # stream-paced start of a projection.
        def par_load(wt, wsrc, xt, xsrc):
            for d in range(DB):
                q0, q1 = (sdma, dma) if d % 2 == 0 else (dma, sdma)
                q0(out=wt[:, d, :], in_=wsrc[:, d, :])
                q1(out=xt[:, d, :], in_=xsrc[d * P : (d + 1) * P, :])

        par_load(wkt_sb, wkt_src, xt_k, kt_d)
        par_load(wvt_sb, wvt_src, xt_v, vt_d)
        # the 1MB of broadcast biases rides behind the K/V streams — bvb
        # is first needed at Vproj's copy-outs, bob not until phase D
        bvb_sb = p_misc.tile([P, D], F32)
        dma(out=bvb_sb[:], in_=bvb_d[:])
        bob_sb = p_misc.tile([P, D], F32)
        sdma(out=bob_sb[:], in_=bob_d[:])
        par_load(wqt_sb, wqt_src, xt_q, qt_d)
        for d in range(DB):
            (sdma if d % 2 == 0 else dma)(
                out=wot_sb[:, d, :], in_=wot_src[:, d, :]
            )

        kt_sb = p_kqt.tile([P, EB, S], BF16)  # K.T: [e_p, e_blk, k]
        qt_sb = p_kqt.tile([P, EB, SQ], BF16)  # Q.T: [e_p, e_blk, q]
        v_sb = p_v.tile([P, KB, D], BF16)  # V:   [k_p, k_blk, e]

        # ---- Phase A1: K projection + per-block staging ----
        # out[e, s] = sum_d W.T[d, e] (stationary) @ _X.T[d, s]; eb-outer
        # so each feature block is complete (and staged for the AllGather)
        # as early as possible.
        # eb-pairs with d-outer inner loop: 4 matmuls consume each arriving
        # (W[d], X[d]) block, so the HBM-fill-paced start of Kproj retires
        # 2x the work per landed block vs an eb-at-a-time sweep.
        def kq_proj(proj_w, proj_x, proj_out, proj_b, stage_to=None):
            for ep in range(EB // 2):
                ps = [
                    p_ps.tile([P, CH], F32, tag="ps", name="ps")
                    for _ in range(2 * QC)
                ]
                for dd in range(DB):
                    for e in range(2):
                        eb = 2 * ep + e
                        for sc in range(QC):
                            nc.tensor.matmul(
                                ps[2 * e + sc][:],
                                proj_w[:, dd, eb * P : (eb + 1) * P],
                                proj_x[:, dd, sc * CH : (sc + 1) * CH],
                                start=(dd == 0),
                                stop=(dd == DB - 1),
                            )
                for e in range(2):
                    eb = 2 * ep + e
                    for sc in range(QC):
                        # DVE, not ACT: ~3x faster per copy-out, frees the
                        # psum slot sooner, and keeps ScalarE clear
                        nc.vector.tensor_scalar_add(
                            proj_out[:, eb, sc * CH : (sc + 1) * CH],
                            ps[2 * e + sc][:],
                            proj_b[:, eb : eb + 1],
                        )
                    if stage_to is not None:
                        sdma(
                            out=stage_to[eb * P : (eb + 1) * P, :],
                            in_=proj_out[:, eb, 0:SH],
                        )

        kq_proj(wkt_sb, xt_k, kt_sb, bk_sb, stage_to=cc_kin)
        nc.gpsimd.collective_compute(
            "AllReduce",
            mybir.AluOpType.add,
            replica_groups=CC_GROUPS,
            ins=[cc_kin.opt()],
            outs=[cc_ksum.opt()],
        )
        # ---- Phase A2: V projection + per-block staging ----
        # V natural: out[k, e] = sum_d _V.T[d, k] (stationary) @ Wv.T[d, e]
        for kb in range(SH // P):
            pse = [
                p_ps.tile([P, CH], F32, tag="ps", name="ps") for _ in range(FC)
            ]
            for dd in range(DB):
                for eh in range(FC):
                    nc.tensor.matmul(
                        pse[eh][:],
                        xt_v[:, dd, kb * P : (kb + 1) * P],
                        wvt_sb[:, dd, eh * CH : (eh + 1) * CH],
                        start=(dd == 0),
                        stop=(dd == DB - 1),
                    )
            for eh in range(FC):
                nc.vector.tensor_add(
                    v_sb[:, kb, eh * CH : (eh + 1) * CH],
                    pse[eh][:],
                    bvb_sb[:, eh * CH : (eh + 1) * CH],
                )
            # V staging on the sync ring (input stream has drained by the
            # time these waits park at its head), keeping ScalarE free.
            dma(out=cc_vin[kb * P : (kb + 1) * P, :], in_=v_sb[:, kb, :])
        # K pair-sum copy-back on the sync ring, behind the V staging: its
        # collective wait parks nothing that anyone needs before phase D.
        # (Not on the scalar ring — a wait there would hold up phase B's
        # exp; ScalarE carries only wait-free weight loads and K staging.)
        for eb in range(EB):
            dma(
                out=kt_sb[:, eb, SH : 2 * SH],
                in_=cc_ksum[eb * P : (eb + 1) * P, :],
            )
        nc.gpsimd.collective_compute(
            "AllReduce",
            mybir.AluOpType.add,
            replica_groups=CC_GROUPS,
            ins=[cc_vin.opt()],
            outs=[cc_vsum.opt()],
        )
        # V pair-sum copy-back on the sync ring: only the phase-D output
        # stores sit behind its collective wait.
        for kb in range(SH // P):
            dma(
                out=v_sb[:, kb + SH // P, :],
                in_=cc_vsum[kb * P : (kb + 1) * P, :],
            )

        # ---- Phase A3: Q projection ----
        kq_proj(wqt_sb, xt_q, qt_sb, bq_sb)

        # Recover the partner halves: other = pair_sum - own. On DVE (3x
        # faster per op than GpSimd — this chain gates B's kb>=8 and C's
        # kb>=8 on slow-collective cores), with explicit wait-until hints:
        # the scheduler's timing model doesn't see collective latency and
        # would otherwise slot these ahead of Qproj's psum copy-outs,
        # parking the DVE queue on the collective for 30us. The hints push
        # them after all Qproj DVE work (K-subs strictly before V-subs —
        # a V-sub queued first would block the earlier-ready K-subs).
        with tc.tile_wait_until(0.15):
            for eb in range(EB):
                nc.vector.tensor_sub(
                    kt_sb[:, eb, SH : 2 * SH],
                    kt_sb[:, eb, SH : 2 * SH],
                    kt_sb[:, eb, 0:SH],
                )
        with tc.tile_wait_until(0.2):
            for kb in range(SH // P):
                nc.vector.tensor_sub(
                    v_sb[:, kb + SH // P, :],
                    v_sb[:, kb + SH // P, :],
                    v_sb[:, kb, :],
                )

        p_wa.release()
        p_es = tc.alloc_tile_pool(name="es", bufs=1, side="right")
        es_sb = p_es.tile([P, KB, SQ], BF16)  # exp(scores): [k_p, k_blk, q]
        s_ps = [
            p_pss.tile([P, CH], F32, tag="sps", name="s_ps") for _ in range(QC)
        ]

        # ---- Phase B: scores[k, q] = K.T' @ Q.T, exp, and key-sums ----
        for kb in range(KB):
            psq = [
                p_ps.tile([P, CH], F32, tag="ps", name="ps") for _ in range(QC)
            ]
            for eb in range(EB):
                for qc in range(QC):
                    nc.tensor.matmul(
                        psq[qc][:],
                        kt_sb[:, eb, kb * P : (kb + 1) * P],
                        qt_sb[:, eb, qc * CH : (qc + 1) * CH],
                        start=(eb == 0),
                        stop=(eb == EB - 1),
                    )
            for qc in range(QC):
                nc.scalar.activation(
                    es_sb[:, kb, qc * CH : (qc + 1) * CH],
                    psq[qc][:],
                    AF.Exp,
                    scale=float(SCALE),
                )
                # sum_k es[k, q], broadcast to every partition row
                nc.tensor.matmul(
                    s_ps[qc][:],
                    ones_sb[:],
                    es_sb[:, kb, qc * CH : (qc + 1) * CH],
                    start=(kb == 0),
                    stop=(kb == KB - 1),
                )
        for qc in range(QC):
            nc.vector.reciprocal(
                recip_sb[:, qc * CH : (qc + 1) * CH], s_ps[qc][:]
            )

        p_kqt.release()
        p_vtu = tc.alloc_tile_pool(name="vtu", bufs=1, side="left")
        vtu_sb = p_vtu.tile([P, EB, SQ], BF16)  # normalized V_.T: [e_p, e_blk, q]

        # ---- Phase C: V_.T[e, q] = (sum_k V[k, e] es[k, q]) * recip[q] ----
        for eb in range(EB):
            psq = [
                p_ps.tile([P, CH], F32, tag="ps", name="ps") for _ in range(QC)
            ]
            for kb in range(KB):
                for qc in range(QC):
                    nc.tensor.matmul(
                        psq[qc][:],
                        v_sb[:, kb, eb * P : (eb + 1) * P],
                        es_sb[:, kb, qc * CH : (qc + 1) * CH],
                        start=(kb == 0),
                        stop=(kb == KB - 1),
                    )
            for qc in range(QC):
                nc.vector.tensor_mul(
                    vtu_sb[:, eb, qc * CH : (qc + 1) * CH],
                    psq[qc][:],
                    recip_sb[:, qc * CH : (qc + 1) * CH],
                )

        p_o = tc.alloc_tile_pool(name="o", bufs=3, side="left")

        # ---- Phase D: O[q, f] = V_.T' @ Wo.T + bo ----
        for qb in range(QB):
            ot = p_o.tile([P, D], F32, tag="ot", name="ot")
            # the last query block's add+store chain is the kernel's tail;
            # quarter-width chunks there shorten the serial add->store->
            # end-barrier dependency
            hc = CH // 2 if qb == QB - 1 else CH
            for fc in range(FC):
                ps = p_ps.tile([P, CH], F32, tag="ps", name="ps")
                for eb in range(EB):
                    nc.tensor.matmul(
                        ps[:],
                        vtu_sb[:, eb, qb * P : (qb + 1) * P],
                        wot_sb[:, eb, fc * CH : (fc + 1) * CH],
                        start=(eb == 0),
                        stop=(eb == EB - 1),
                    )
                for h in range(CH // hc):
                    nc.vector.tensor_add(
                        ot[:, fc * CH + h * hc : fc * CH + (h + 1) * hc],
                        ps[:, h * hc : (h + 1) * hc],
                        bob_sb[:, fc * CH + h * hc : fc * CH + (h + 1) * hc],
                    )
            # per-chunk stores so the first half ships while the second
            # half's add is still running
            for fc in range(FC):
                for h in range(CH // hc):
                    dma(
                        out=o_d[
                            qb * P : (qb + 1) * P,
                            fc * CH + h * hc : fc * CH + (h + 1) * hc,
                        ],
                        in_=ot[:, fc * CH + h * hc : fc * CH + (h + 1) * hc],
                    )

        p_es.release()
        p_xv.release()
        p_xq.release()
        p_xk.release()
        p_v.release()
        p_o.release()
        p_vtu.release()
        p_wo.release()
        p_misc.release()
        p_dram.release()
        p_pss.release()
        p_ps.release()

    nc.finalize()
    return nc


def get_nc() -> bass.Bass:
    global _NC_CACHE
    if _NC_CACHE is None:
        _NC_CACHE = _build_nc()
    return _NC_CACHE


def make_in_maps(inputs: dict) -> list[dict]:
    _K = np.asarray(inputs["_K"], dtype=np.float32)
    _V = np.asarray(inputs["_V"], dtype=np.float32)
    _Q = np.asarray(inputs["_Q"], dtype=np.float32)

    shared = {
        "wkt": np.ascontiguousarray(
            np.asarray(inputs["Wk"], np.float32).T.astype(NPBF16)
        ),
        "wqt": np.ascontiguousarray(
            np.asarray(inputs["Wq"], np.float32).T.astype(NPBF16)
        ),
        "wvt": np.ascontiguousarray(
            np.asarray(inputs["Wv"], np.float32).T.astype(NPBF16)
        ),
        "wot": np.ascontiguousarray(
            np.asarray(inputs["Wo"], np.float32).T.astype(NPBF16)
        ),
        "bk": np.ascontiguousarray(
            np.asarray(inputs["bk"], np.float32).reshape(EB, P).T
        ),
        "bq": np.ascontiguousarray(
            np.asarray(inputs["bq"], np.float32).reshape(EB, P).T
        ),
        "bvb": np.ascontiguousarray(
            np.broadcast_to(np.asarray(inputs["bv"], np.float32), (P, D))
        ),
        "bob": np.ascontiguousarray(
            np.broadcast_to(np.asarray(inputs["bo"], np.float32), (P, D))
        ),
    }

    in_maps = []
    for c in range(8):
        b, h = divmod(c, 2)
        # Each core projects its own key half (h picks it: pair rank order
        # matches k order) and its own query half.
        kt = np.ascontiguousarray(
            _K[b, h * SH : (h + 1) * SH, :].T.astype(NPBF16)
        )
        vt = np.ascontiguousarray(
            _V[b, h * SH : (h + 1) * SH, :].T.astype(NPBF16)
        )
        qt = np.ascontiguousarray(
            _Q[b, h * SQ : (h + 1) * SQ, :].T.astype(NPBF16)
        )
        in_maps.append({"kt": kt, "vt": vt, "qt": qt, **shared})
    return in_maps


def kernel(**inputs) -> np.ndarray:
    global LAST_EXEC_NS
    nc = get_nc()
    in_maps = make_in_maps(inputs)
    kwargs = {}
    if TRACE and TRACE_ALL_CORES:
        kwargs["trace_cores"] = list(range(8))
    if TRACE and TRACE_TMPDIR:
        kwargs["tmpdir"] = TRACE_TMPDIR
    res = run_bass_kernel_spmd(
        nc, in_maps, core_ids=list(range(8)), trace=TRACE, **kwargs
    )
    LAST_EXEC_NS = res.exec_time_ns

    out = np.empty((B, S, D), dtype=np.float32)
    for c in range(8):
        b, h = divmod(c, 2)
        out[b, h * SQ : (h + 1) * SQ, :] = res.results[c]["o"]
    return out
